# revision 16
# baseline (speedup 1.0000x reference)
"""Trainium2 Bass kernel for nn_Cifp_48206712930739 (topk_masking head), v4.

Column-parallel classification head over 8 NeuronCores: each core owns
C/8 = 12500 classes (a [512, 12500] slice of the kernel matrix) and the
embeddings are replicated.

v4 (vs v3):
  * tgt is computed EARLY from a host-gathered [512, 256] matrix of the
    owned labels' kernel columns, pushed through the exact same
    cast/norm/matmul pipeline as the main sweep (so the value is
    bit-identical to the sweep's label element), and AllReduced while
    the sweep runs.  The per-chunk iota extraction is gone.
  * phase D (count / shift / seg-max8) is fused INTO the main sweep:
    each 500-col chunk is counted (is_ge, so the label element is
    evicted), shifted into a scratch tile and max8-reduced right after
    its cos block is produced.  No resident [128, 25600] tile at all.
  * column rsqrt via exp(-0.5*ln(x)) on ScalarE instead of the slow DVE
    reciprocal; kernel chunks are loaded with an f32->bf16 casting
    SWDGE DMA (no Act copy).
  * the gpsimd kth_largest (~178us fixed cost) is replaced by an exact
    rank-count selection: the top-4 per transposed partition-half
    ([128, 8] = 1024 values, provably containing the global top-400)
    are broadcast to all partitions via a matmul, each candidate's
    global rank is computed with 8 small is_gt count ops, and th is
    extracted as the unique candidate whose rank equals far_rank-1.
    All values > th are in the gathered candidate tile, so per-row
    moments come from [128, 256] slices; no final AllReduce.

Self-contained: hardcodes all shapes from the problem spec.
"""

import ml_dtypes
import numpy as np

import concourse.bass as bass
import concourse.bacc as bacc
import concourse.mybir as mybir
import concourse.tile as tile
from concourse import bass_utils, library_config
from contextlib import ExitStack

F32 = mybir.dt.float32
BF16 = mybir.dt.bfloat16
I32 = mybir.dt.int32
AF = mybir.ActivationFunctionType
OP = mybir.AluOpType
AX = mybir.AxisListType

P = 128
N, D, C = 256, 512, 100000
NCORE = 8
CLOC = C // NCORE            # 12500 classes per core
CH = 500                     # sweep column chunk (<=512 for PSUM f32)
NCH = CLOC // CH             # 25
NEG_PAD = -1.0e30
BIGOFF = 1 << 23             # out-of-bounds marker for unowned rows
SCALE = 64.0
MARGIN = 0.35
M2 = 2.0 * SCALE             # 128 = scaled "-2" mask shift
NC1 = N * (C - 1)            # 25599744
# candidate pipeline sizes
LTOP = 32                    # sorted top-32 kept per (partition, group)
GW = 2 * LTOP                # 64 candidate slots per core in the gather
GALL = NCORE * GW            # 512 gathered slots per partition
AGW = P * GW + 8             # AG payload: 8192 candidates + count + pad
RW = 1024                    # replicated rank set: 8 slots x 128 parts


def build(n_iter=1):
    nc = bacc.Bacc("TRN2", target_bir_lowering=False, debug=False,
                   enable_asserts=True, num_devices=NCORE)

    emb_in = nc.dram_tensor("embeddings", [N, D], F32, kind="ExternalInput")
    ker_in = nc.dram_tensor("kers", [D, CLOC], BF16, kind="ExternalInput")
    kg_in = nc.dram_tensor("kg", [D, N], BF16, kind="ExternalInput")
    offs_in = nc.dram_tensor("offs", [P, 2], I32, kind="ExternalInput")
    ctab_in = nc.dram_tensor("ctab", [16, 32], F32, kind="ExternalInput")
    eye_in = nc.dram_tensor("eye", [P, P], F32, kind="ExternalInput")
    iota_in = nc.dram_tensor("iotaf", [P, CH], F32, kind="ExternalInput")
    labv_in = nc.dram_tensor("labv", [P, 2], F32, kind="ExternalInput")
    rowid_in = nc.dram_tensor("rowid", [P, 2], F32, kind="ExternalInput")

    out1 = nc.dram_tensor("out1", [N, CLOC], F32, kind="ExternalOutput")
    out2 = nc.dram_tensor("out2", [N, CLOC], F32, kind="ExternalOutput")
    dbg = nc.dram_tensor("dbg", [P, 16], F32, kind="ExternalOutput")

    arb_i = nc.dram_tensor("arb_i", [P, 2], F32, kind="Internal")
    arb_o = nc.dram_tensor("arb_o", [P, 2], F32, kind="Internal",
                           addr_space="Shared")
    agc_i = nc.dram_tensor("agc_i", [1, AGW], F32, kind="Internal")
    agc_o = nc.dram_tensor("agc_o", [NCORE, AGW], F32, kind="Internal",
                           addr_space="Shared")
    rb_d = nc.dram_tensor("rb_d", [1, RW], F32, kind="Internal")

    rg = [list(range(NCORE))]
    out1_flat = out1.ap().rearrange("a (b o) -> (a b) o", o=1)
    kers_r = ker_in.ap().rearrange("(k p) c -> p k c", p=P)  # [128,4,CLOC]
    kg_r = kg_in.ap().rearrange("(k p) c -> p k c", p=P)     # [128,4,256]

    with tile.TileContext(nc) as tc:
        _emit(nc, tc, emb_in, offs_in, ctab_in, eye_in, iota_in, labv_in,
              rowid_in, kers_r, kg_r, out1, out2, dbg,
              arb_i, arb_o, agc_i, agc_o, rb_d, rg, out1_flat)

    nc.compile()
    return nc


def _emit(nc, tc, emb_in, offs_in, ctab_in, eye_in, iota_in, labv_in,
          rowid_in, kers_r, kg_r, out1, out2, dbg,
          arb_i, arb_o, agc_i, agc_o, rb_d, rg, out1_flat):
    with ExitStack() as top:
        cp = top.enter_context(tc.tile_pool(name="const", bufs=1))
        eye = cp.tile([P, P], F32)
        nc.sync.dma_start(eye[:], eye_in.ap())
        ones_k = cp.tile([P, 1], F32)
        nc.vector.memset(ones_k[:], 1.0)
        ones_r = cp.tile([1, P], F32)
        nc.vector.memset(ones_r[:], 1.0)
        ones_kb = cp.tile([P, 1], BF16)
        nc.vector.memset(ones_kb[:], 1.0)
        ones_rb = cp.tile([1, P], BF16)
        nc.vector.memset(ones_rb[:], 1.0)
        ones128b = cp.tile([P, P], BF16)
        nc.vector.memset(ones128b[:], 1.0)
        ctab = cp.tile([16, 32], F32)
        nc.sync.dma_start(ctab[:], ctab_in.ap())
        iotaf = cp.tile([P, CH], F32)
        nc.sync.dma_start(iotaf[:], iota_in.ap())
        labv = cp.tile([P, 2], F32)
        nc.sync.dma_start(labv[:], labv_in.ap())
        rowid = cp.tile([P, 2], F32)
        nc.sync.dma_start(rowid[:], rowid_in.ap())
        offs = cp.tile([P, 2], I32)
        nc.sync.dma_start(offs[:], offs_in.ap())
        embT = [cp.tile([P, N], BF16, name=f"embT_{k}") for k in range(4)]

        # ---------------- phase A: embedding prep --------------------------
        with ExitStack() as s0:
            pp = s0.enter_context(tc.tile_pool(name="prep", bufs=2))
            pps = s0.enter_context(tc.tile_pool(name="prepps", bufs=2,
                                                space="PSUM"))
            for g in range(2):
                et = pp.tile([P, D], F32, tag="et")
                nc.sync.dma_start(et[:], emb_in.ap()[g * P:(g + 1) * P, :])
                sscr = pp.tile([P, D], F32, tag="sscr")
                n2 = pp.tile([P, 1], F32, tag="n2")
                nc.scalar.activation(sscr[:], et[:], AF.Square,
                                     accum_out=n2[:])
                rinv = pp.tile([P, 1], F32, tag="rinv")
                nc.scalar.activation(rinv[:], n2[:], AF.Abs_reciprocal_sqrt)
                et64 = pp.tile([P, D], F32, tag="et64")
                nc.vector.tensor_scalar(et64[:], et[:], rinv[:, :1], SCALE,
                                        op0=OP.mult, op1=OP.mult)
                for k in range(4):
                    pt = pps.tile([P, P], F32)
                    nc.tensor.transpose(pt[:], et64[:, k * P:(k + 1) * P],
                                        eye[:])
                    nc.scalar.activation(
                        embT[k][:, g * P:(g + 1) * P], pt[:], AF.Copy)

        # ---------------- phase A2: early tgt from label columns -----------
        # kg holds, for every row, the kernel column of its label if this
        # core owns it (else a dummy).  Run it through the exact pipeline
        # of the main sweep so the extracted value is bit-identical to the
        # sweep's label element, then AllReduce (overlaps the sweep).
        tgt = cp.tile([P, 2], F32)
        with ExitStack() as sA:
            ap_ = sA.enter_context(tc.tile_pool(name="a2", bufs=1))
            aps = sA.enter_context(tc.tile_pool(name="a2ps", bufs=2,
                                                space="PSUM"))
            kgb = ap_.tile([P, 4, N], BF16)
            nc.sync.dma_start(kgb[:], kg_r[:, :, :])
            sqg = ap_.tile([P, 4, N], BF16, tag="sqg")
            nc.scalar.activation(sqg[:], kgb[:], AF.Square)
            ksg = ap_.tile([P, 2, N], BF16, tag="ksg")
            nc.vector.tensor_tensor(ksg[:, 0, :], sqg[:, 0, :], sqg[:, 1, :],
                                    OP.add)
            nc.vector.tensor_tensor(ksg[:, 1, :], sqg[:, 2, :], sqg[:, 3, :],
                                    OP.add)
            ksgs = ap_.tile([P, N], BF16, tag="ksgs")
            nc.vector.tensor_tensor(ksgs[:], ksg[:, 0, :], ksg[:, 1, :],
                                    OP.add)
            png = aps.tile([P, N], F32)
            nc.tensor.matmul(png[:], ones128b[:], ksgs[:],
                             start=True, stop=True)
            rsg = ap_.tile([P, N], F32, tag="rsg")
            nc.scalar.activation(rsg[:], png[:], AF.Abs_reciprocal_sqrt)
            tloc = ap_.tile([P, 2], F32, tag="tloc")
            for m in range(2):
                pcg = aps.tile([P, N], F32, tag="pcg")
                for k in range(4):
                    nc.tensor.matmul(pcg[:], embT[k][:, m * P:(m + 1) * P],
                                     kgb[:, k, :], start=(k == 0),
                                     stop=(k == 3))
                slg = ap_.tile([P, N], F32, tag="slg")
                nc.vector.tensor_tensor(slg[:], pcg[:], rsg[:], OP.mult)
                scr = ap_.tile([P, N], F32, tag="scrg")
                nc.vector.scalar_tensor_tensor(
                    scr[:], iotaf[:, 0:N], rowid[:, m:m + 1], slg[:],
                    op0=OP.is_equal, op1=OP.mult,
                    accum_out=tloc[:, m:m + 1])
            nc.sync.dma_start(arb_i.ap(), tloc[:])
            nc.gpsimd.collective_compute(
                "AllReduce", OP.add, replica_groups=rg,
                ins=[arb_i.ap()], outs=[arb_o.ap()])
            nc.sync.dma_start(tgt[:], arb_o.ap())
        negtgt = cp.tile([P, 2], F32)
        nc.vector.tensor_scalar(negtgt[:], tgt[:], -1.0, None, op0=OP.mult)

        # ---------------- phase B: fused main sweep ------------------------
        # per chunk: cast-load kernel cols, col rsqrt via ln/exp, cos
        # matmul, write both outputs, then count/shift/max8 immediately.
        cnt = cp.tile([P, 2, NCH], F32)
        cand0 = cp.tile([P, 2, NCH * 8], F32)
        with ExitStack() as s1:
            kp = s1.enter_context(tc.tile_pool(name="kt", bufs=3))
            sqp = s1.enter_context(tc.tile_pool(name="sq", bufs=3))
            lnp = s1.enter_context(tc.tile_pool(name="ln", bufs=3))
            rsp = s1.enter_context(tc.tile_pool(name="rs", bufs=3))
            slp = s1.enter_context(tc.tile_pool(name="sl", bufs=4))
            msp = s1.enter_context(tc.tile_pool(name="msh", bufs=3))
            pcp = s1.enter_context(tc.tile_pool(name="pc", bufs=6,
                                                space="PSUM"))
            pnp = s1.enter_context(tc.tile_pool(name="pn", bufs=2,
                                                space="PSUM"))
            for ci in range(NCH):
                c0 = ci * CH
                ktb = kp.tile([P, 4, CH], BF16)
                nc.sync.dma_start(ktb[:], kers_r[:, :, c0:c0 + CH])
                pcs = []
                for m in range(2):
                    pcos = pcp.tile([P, CH], F32)
                    for k in range(4):
                        nc.tensor.matmul(pcos[:],
                                         embT[k][:, m * P:(m + 1) * P],
                                         ktb[:, k, :],
                                         start=(k == 0), stop=(k == 3))
                    pcs.append(pcos)
                sqt = sqp.tile([P, 4, CH], BF16)
                nc.scalar.activation(sqt[:], ktb[:], AF.Square)
                ks2 = sqp.tile([P, 2, CH], BF16, tag="ks2")
                nc.gpsimd.tensor_tensor(ks2[:, 0, :], sqt[:, 0, :],
                                        sqt[:, 1, :], OP.add)
                nc.gpsimd.tensor_tensor(ks2[:, 1, :], sqt[:, 2, :],
                                        sqt[:, 3, :], OP.add)
                ks = sqp.tile([P, CH], BF16, tag="ks")
                nc.gpsimd.tensor_tensor(ks[:], ks2[:, 0, :], ks2[:, 1, :],
                                        OP.add)
                pnrm = pnp.tile([P, CH], F32)
                nc.tensor.matmul(pnrm[:], ones128b[:], ks[:],
                                 start=True, stop=True)
                rs = rsp.tile([P, CH], F32)
                nc.scalar.activation(rs[:], pnrm[:], AF.Abs_reciprocal_sqrt)
                sl = slp.tile([P, 2, CH], F32)
                sg = msp.tile([P, 2, CH], BF16, tag="sg")
                for m in range(2):
                    pcos = pcs[m]
                    nc.vector.tensor_tensor(sl[:, m, :], pcos[:], rs[:],
                                            OP.mult)
                    nc.sync.dma_start(
                        out2.ap()[m * P:(m + 1) * P, c0:c0 + CH], sl[:, m, :])
                    nc.sync.dma_start(
                        out1.ap()[m * P:(m + 1) * P, c0:c0 + CH], sl[:, m, :])
                    nc.scalar.activation(sg[:, m, :], sl[:, m, :], AF.Sign,
                                         bias=negtgt[:, m:m + 1],
                                         accum_out=cnt[:, m, ci:ci + 1])
                sh = msp.tile([P, 2, CH], F32, tag="sh")
                nc.vector.scalar_tensor_tensor(
                    sh[:], sg[:], 0.0, sl[:], op0=OP.is_lt, op1=OP.mult)
                for m in range(2):
                    nc.vector.max(cand0[:, m, ci * 8:ci * 8 + 8],
                                  sh[:, m, :])

        sm = top.enter_context(tc.tile_pool(name="small", bufs=1))
        sps = top.enter_context(tc.tile_pool(name="smallps", bufs=1,
                                             space="PSUM"))

        # ---------------- phase E: counts + sorted top-32 + AG -------------
        # count_gt = count_ge - #owned labels (evicted bit-exactly)
        own = sm.tile([P, 2], F32)
        nc.vector.tensor_scalar(own[:], labv[:], -0.5, None, op0=OP.is_gt)
        owns = sm.tile([P, 1], F32)
        nc.vector.tensor_reduce(owns[:], own[:], AX.X, OP.add)
        cntr = sm.tile([P, 1], F32)
        nc.vector.tensor_reduce(cntr[:], cnt[:].rearrange("p a b -> p (a b)"),
                                AX.X, OP.add)
        nc.vector.tensor_scalar(cntr[:], cntr[:], float(2 * CLOC), None,
                                op0=OP.add)
        nc.vector.tensor_tensor(cntr[:], cntr[:], owns[:], OP.subtract)
        nc.vector.tensor_scalar(cntr[:], cntr[:], 0.5, None, op0=OP.mult)
        pcnt = sps.tile([1, 1], F32, tag="pcnt")
        nc.tensor.matmul(pcnt[:], cntr[:], ones_k[:], start=True, stop=True)
        cnts = sm.tile([1, 1], F32)
        nc.scalar.activation(cnts[:], pcnt[:], AF.Copy)

        # per (partition, group): 4 rounds of max8 + match_replace over the
        # 200 chunk-candidates -> sorted top-32 list L
        L = sm.tile([P, 2, LTOP], F32)
        for g in range(2):
            blk = cand0[:, g, :]
            for r in range(4):
                nc.vector.max(L[:, g, 8 * r:8 * (r + 1)], blk)
                if r < 3:
                    nc.vector.match_replace(blk, L[:, g, 8 * r:8 * (r + 1)],
                                            blk, NEG_PAD)

        # ship L + count partial in one AllGather
        lflat = agc_i.ap()[0:1, 0:P * GW].rearrange("o (p f) -> (o p) f", p=P)
        nc.sync.dma_start(lflat, L[:].rearrange("p a b -> p (a b)"))
        nc.sync.dma_start(agc_i.ap()[0:1, P * GW:P * GW + 1], cnts[:])
        nc.gpsimd.collective_compute(
            "AllGather", OP.bypass, replica_groups=rg,
            ins=[agc_i.ap()], outs=[agc_o.ap()])

        # ---------------- phase F: gather back, far_rank, transpose --------
        G = sm.tile([P, GALL], F32)          # col r*64 + g*32 + j
        for r in range(NCORE):
            blk = agc_o.ap()[r:r + 1, 0:P * GW].rearrange(
                "o (p f) -> (o p) f", p=P)
            nc.sync.dma_start(G[:, r * GW:(r + 1) * GW], blk)
        cntg = sm.tile([1, NCORE], F32)
        nc.sync.dma_start(cntg[:], agc_o.ap()[:, P * GW:P * GW + 1]
                          .rearrange("a o -> o a"))
        tsum = sm.tile([1, 1], F32)
        nc.vector.tensor_reduce(tsum[:], cntg[:], AX.X, OP.add)
        a_t = sm.tile([1, 1], F32)
        nc.vector.tensor_scalar(a_t[:], tsum[:], -1.0, float(NC1),
                                op0=OP.mult, op1=OP.add)

        # k_idx = clip(far_rank - 1, 0, 255) via counting 99999*k < A
        pa16 = sps.tile([16, 1], F32, tag="pa16")
        nc.tensor.matmul(pa16[:], ones_r[0:1, 0:16], a_t[:],
                         start=True, stop=True)
        a16 = sm.tile([16, 1], F32)
        nc.scalar.activation(a16[:], pa16[:], AF.Copy)
        kscr = sm.tile([16, 16], F32)
        kpart = sm.tile([16, 1], F32)
        nc.vector.tensor_scalar(kscr[:], ctab[:, 0:16], a16[:, :1], None,
                                op0=OP.is_lt, op1=OP.add,
                                accum_out=kpart[:])
        pki = sps.tile([1, 1], F32, tag="pki")
        nc.tensor.matmul(pki[:], kpart[:], ones_k[0:16, :],
                         start=True, stop=True)
        ki = sm.tile([1, 1], F32)
        nc.scalar.activation(ki[:], pki[:], AF.Copy)
        pki128 = sps.tile([P, 1], F32, tag="pki128")
        nc.tensor.matmul(pki128[:], ones_r[:], ki[:], start=True, stop=True)
        ki128 = sm.tile([P, 1], F32)
        nc.scalar.activation(ki128[:], pki128[:], AF.Copy)

        # transpose G so sorted-rank slots become partitions
        T = sm.tile([P, GALL], F32)
        with ExitStack() as s3:
            tps = s3.enter_context(tc.tile_pool(name="tp", bufs=2,
                                                space="PSUM"))
            for b in range(4):
                pt = tps.tile([P, P], F32)
                nc.tensor.transpose(pt[:], G[:, b * P:(b + 1) * P], eye[:])
                nc.scalar.activation(T[:, b * P:(b + 1) * P], pt[:], AF.Copy)

        # final candidates: top-8 of each transposed half (cores 0-3 / 4-7)
        fin = sm.tile([P, 2, 8], F32)
        for h in range(2):
            nc.vector.max(fin[:, h, :], T[:, 256 * h:256 * (h + 1)])

        # ---------------- phase G: exact th via rank counting --------------
        # top-4 of each half per partition = 1024 values containing the
        # global top-400.  Broadcast them to every partition, rank each by
        # an is_gt count, and select the one whose rank == k_idx.
        f8c = sm.tile([P, 2, 4], F32)            # contiguous top-4 per half
        nc.vector.tensor_scalar(f8c[:], fin[:, :, 0:4], 0.0, None, op0=OP.add)
        rb_flat = rb_d.ap()[0:1, :].rearrange("o (p f) -> (o p) f", p=P)
        nc.sync.dma_start(rb_flat, f8c[:].rearrange("p a b -> p (a b)"))
        row = sm.tile([1, RW], F32)
        nc.sync.dma_start(row[:], rb_d.ap())
        R = sm.tile([P, RW], F32)
        with ExitStack() as s4:
            prp = s4.enter_context(tc.tile_pool(name="prp", bufs=1,
                                                space="PSUM"))
            pR = prp.tile([P, RW], F32)
            for hh in range(2):
                nc.tensor.matmul(pR[:, hh * 512:(hh + 1) * 512], ones_r[:],
                                 row[:, hh * 512:(hh + 1) * 512],
                                 start=True, stop=True)
            nc.scalar.activation(R[:], pR[:], AF.Copy)
        rnk = sm.tile([P, 2, 4], F32)
        rscr = sm.tile([P, RW], F32)
        for h in range(2):
            for j in range(4):
                nc.vector.tensor_scalar(rscr[:], R[:], f8c[:, h, j:j + 1],
                                        None, op0=OP.is_gt, op1=OP.add,
                                        accum_out=rnk[:, h, j:j + 1])
        m8 = sm.tile([P, 2, 4], F32)
        nc.vector.tensor_scalar(m8[:], rnk[:], ki128[:, :1], None,
                                op0=OP.is_equal)
        selv = sm.tile([P, 2, 4], F32)
        nc.vector.tensor_tensor(selv[:], m8[:], f8c[:], OP.mult)
        thp = sm.tile([P, 1], F32)
        nc.vector.tensor_reduce(thp[:], selv[:].rearrange("p a b -> p (a b)"),
                                AX.X, OP.add)
        pth = sps.tile([1, 1], F32, tag="pth")
        nc.tensor.matmul(pth[:], thp[:], ones_k[:], start=True, stop=True)
        th1 = sm.tile([1, 1], F32)
        nc.scalar.activation(th1[:], pth[:], AF.Copy)
        pth128 = sps.tile([P, 1], F32, tag="pth128")
        nc.tensor.matmul(pth128[:], ones_r[:], th1[:], start=True, stop=True)
        th128 = sm.tile([P, 1], F32)
        nc.scalar.activation(th128[:], pth128[:], AF.Copy)

        # ---------------- phase H: moments from candidates -----------------
        sq = sm.tile([P, 2], F32)
        tm = sm.tile([P, 2], F32)
        wsc = sm.tile([P, GALL], F32)
        w2 = sm.tile([P, GALL], F32)
        for g in range(2):
            Gg = G[:].rearrange("p (r g j) -> p g r j", g=2, j=LTOP)[:, g, :, :]
            wv = wsc[:].rearrange("p (r g j) -> p g r j",
                                  g=2, j=LTOP)[:, g, :, :]
            w2v = w2[:].rearrange("p (r g j) -> p g r j",
                                  g=2, j=LTOP)[:, g, :, :]
            nc.vector.scalar_tensor_tensor(
                wv, Gg, th128[:, :1], Gg, op0=OP.is_gt, op1=OP.mult)
            nc.scalar.activation(w2v, wv, AF.Square, accum_out=sq[:, g:g + 1])
            nc.vector.tensor_scalar(wv, Gg, th128[:, :1], None,
                                    op0=OP.is_gt, op1=OP.add,
                                    accum_out=tm[:, g:g + 1])

        # ---------------- phase I: final scalar math + patch out1 ----------
        times = sm.tile([P, 2], F32)
        nc.vector.tensor_scalar(times[:], tm[:], 1.0, None, op0=OP.max)
        rec = sm.tile([P, 2], F32)
        nc.vector.reciprocal(rec[:], times[:])
        nm = sm.tile([P, 2], F32)
        nc.vector.tensor_tensor(nm[:], sq[:], rec[:], OP.mult)
        nc.vector.tensor_scalar(nm[:], nm[:], 1.0 / (SCALE * SCALE), None,
                                op0=OP.mult)
        x5 = sm.tile([P, 2], F32)
        nc.vector.tensor_scalar(x5[:], tgt[:], SCALE, None, op0=OP.add)
        x6 = sm.tile([P, 2], F32)
        nc.vector.tensor_tensor(x6[:], x5[:], nm[:], OP.mult)
        pv2 = sm.tile([P, 2], F32)
        nc.vector.tensor_tensor(pv2[:], tgt[:], x6[:], OP.subtract)
        nc.vector.tensor_scalar(pv2[:], pv2[:], -SCALE * MARGIN, None,
                                op0=OP.add)
        for g in range(2):
            nc.gpsimd.indirect_dma_start(
                out=out1_flat,
                out_offset=bass.IndirectOffsetOnAxis(ap=offs[:, g:g + 1],
                                                     axis=0),
                in_=pv2[:, g:g + 1], in_offset=None,
                bounds_check=N * CLOC - 1, oob_is_err=False)

        nc.sync.dma_start(dbg.ap()[:, 0:2], sq[:])
        nc.sync.dma_start(dbg.ap()[:, 2:4], tm[:])
        nc.sync.dma_start(dbg.ap()[:, 6:7], th128[:])
        nc.sync.dma_start(dbg.ap()[:, 7:9], nm[:])
        nc.sync.dma_start(dbg.ap()[:, 11:13], pv2[:])
        nc.sync.dma_start(dbg.ap()[:, 13:15], tgt[:])
        nc.sync.dma_start(dbg.ap()[0:1, 15:16], ki[0:1, :])
        nc.sync.dma_start(dbg.ap()[0:1, 4:5], cnts[:])


_NC = None


def _get_nc():
    global _NC
    if _NC is None:
        _NC = build()
    return _NC


def _make_in_maps(embeddings, kernel, label):
    emb = np.ascontiguousarray(np.asarray(embeddings, dtype=np.float32))
    ker = np.asarray(kernel, dtype=np.float32)
    lab = np.asarray(label).astype(np.int64)

    ctab = np.zeros((16, 32), np.float32)
    kk = (np.arange(16)[:, None] * 16 + np.arange(16)[None, :])
    ctab[:, :16] = (float(C - 1) * kk).astype(np.float32)
    ctab[0, 0] = 1.0e30
    ctab[:, 16:] = kk.astype(np.float32)
    eye = np.eye(P, dtype=np.float32)
    iotaf = np.tile(np.arange(CH, dtype=np.float32), (P, 1))

    rows = np.arange(N)
    in_maps = []
    for c in range(NCORE):
        loc = lab - c * CLOC
        owned = (loc >= 0) & (loc < CLOC)
        off = np.where(owned, rows * CLOC + loc, BIGOFF).astype(np.int32)
        offs = off.reshape(2, P).T.copy()  # [128, 2]: row i = p + 128*g
        labv = np.where(owned, loc, -5.0).astype(np.float32)
        labv = labv.reshape(2, P).T.copy()  # [128, 2]
        rowid = np.where(owned, rows, -5.0).astype(np.float32)
        rowid = rowid.reshape(2, P).T.copy()  # [128, 2]
        kslice = np.ascontiguousarray(
            ker[:, c * CLOC:(c + 1) * CLOC].astype(ml_dtypes.bfloat16))
        kg = kslice[:, np.where(owned, loc, 0)]  # [512, 256] bf16
        in_maps.append({
            "embeddings": emb,
            "kers": kslice,
            "kg": np.ascontiguousarray(kg),
            "offs": offs,
            "ctab": ctab,
            "eye": eye,
            "iotaf": iotaf,
            "labv": labv,
            "rowid": rowid,
        })
    return in_maps


def run(embeddings, kernel, label, trace=False):
    nc = _get_nc()
    in_maps = _make_in_maps(embeddings, kernel, label)
    res = bass_utils.run_bass_kernel_spmd(
        nc, in_maps, core_ids=list(range(NCORE)), trace=trace)
    out1 = np.concatenate([res.results[c]["out1"] for c in range(NCORE)],
                          axis=1)
    out2 = np.concatenate([res.results[c]["out2"] for c in range(NCORE)],
                          axis=1)
    return (out1, out2), res


def kernel(**inputs):
    outs, _ = run(inputs["embeddings"], inputs["kernel"], inputs["label"])
    return outs


# revision 17
# speedup vs baseline: 1.0075x; 1.0075x over previous
"""Trainium2 Bass kernel for nn_Cifp_48206712930739 (topk_masking head), v4.

Column-parallel classification head over 8 NeuronCores: each core owns
C/8 = 12500 classes (a [512, 12500] slice of the kernel matrix) and the
embeddings are replicated.

v4 (vs v3):
  * tgt is computed EARLY from a host-gathered [512, 256] matrix of the
    owned labels' kernel columns, pushed through the exact same
    cast/norm/matmul pipeline as the main sweep (so the value is
    bit-identical to the sweep's label element), and AllReduced while
    the sweep runs.  The per-chunk iota extraction is gone.
  * phase D (count / shift / seg-max8) is fused INTO the main sweep:
    each 500-col chunk is counted (is_ge, so the label element is
    evicted), shifted into a scratch tile and max8-reduced right after
    its cos block is produced.  No resident [128, 25600] tile at all.
  * column rsqrt via exp(-0.5*ln(x)) on ScalarE instead of the slow DVE
    reciprocal; kernel chunks are loaded with an f32->bf16 casting
    SWDGE DMA (no Act copy).
  * the gpsimd kth_largest (~178us fixed cost) is replaced by an exact
    rank-count selection: the top-4 per transposed partition-half
    ([128, 8] = 1024 values, provably containing the global top-400)
    are broadcast to all partitions via a matmul, each candidate's
    global rank is computed with 8 small is_gt count ops, and th is
    extracted as the unique candidate whose rank equals far_rank-1.
    All values > th are in the gathered candidate tile, so per-row
    moments come from [128, 256] slices; no final AllReduce.

Self-contained: hardcodes all shapes from the problem spec.
"""

import ml_dtypes
import numpy as np

import concourse.bass as bass
import concourse.bacc as bacc
import concourse.mybir as mybir
import concourse.tile as tile
from concourse import bass_utils, library_config
from contextlib import ExitStack

F32 = mybir.dt.float32
BF16 = mybir.dt.bfloat16
I32 = mybir.dt.int32
AF = mybir.ActivationFunctionType
OP = mybir.AluOpType
AX = mybir.AxisListType

P = 128
N, D, C = 256, 512, 100000
NCORE = 8
CLOC = C // NCORE            # 12500 classes per core
CH = 500                     # sweep column chunk (<=512 for PSUM f32)
NCH = CLOC // CH             # 25
NEG_PAD = -1.0e30
BIGOFF = 1 << 23             # out-of-bounds marker for unowned rows
SCALE = 64.0
MARGIN = 0.35
M2 = 2.0 * SCALE             # 128 = scaled "-2" mask shift
NC1 = N * (C - 1)            # 25599744
# candidate pipeline sizes
LTOP = 32                    # sorted top-32 kept per (partition, group)
GW = 2 * LTOP                # 64 candidate slots per core in the gather
GALL = NCORE * GW            # 512 gathered slots per partition
AGW = P * GW + 8             # AG payload: 8192 candidates + count + pad
RW = 1024                    # replicated rank set: 8 slots x 128 parts


def build(n_iter=1):
    nc = bacc.Bacc("TRN2", target_bir_lowering=False, debug=False,
                   enable_asserts=True, num_devices=NCORE)

    emb_in = nc.dram_tensor("embeddings", [N, D], F32, kind="ExternalInput")
    ker_in = nc.dram_tensor("kers", [D, CLOC], BF16, kind="ExternalInput")
    kg_in = nc.dram_tensor("kg", [D, N], BF16, kind="ExternalInput")
    offs_in = nc.dram_tensor("offs", [P, 2], I32, kind="ExternalInput")
    ctab_in = nc.dram_tensor("ctab", [16, 32], F32, kind="ExternalInput")
    eye_in = nc.dram_tensor("eye", [P, P], F32, kind="ExternalInput")
    iota_in = nc.dram_tensor("iotaf", [P, CH], F32, kind="ExternalInput")
    labv_in = nc.dram_tensor("labv", [P, 2], F32, kind="ExternalInput")
    rowid_in = nc.dram_tensor("rowid", [P, 2], F32, kind="ExternalInput")

    out1 = nc.dram_tensor("out1", [N, CLOC], F32, kind="ExternalOutput")
    out2 = nc.dram_tensor("out2", [N, CLOC], F32, kind="ExternalOutput")
    dbg = nc.dram_tensor("dbg", [P, 16], F32, kind="ExternalOutput")

    arb_i = nc.dram_tensor("arb_i", [P, 2], F32, kind="Internal")
    arb_o = nc.dram_tensor("arb_o", [P, 2], F32, kind="Internal",
                           addr_space="Shared")
    agc_i = nc.dram_tensor("agc_i", [1, AGW], F32, kind="Internal")
    agc_o = nc.dram_tensor("agc_o", [NCORE, AGW], F32, kind="Internal",
                           addr_space="Shared")
    rb_d = nc.dram_tensor("rb_d", [1, RW], F32, kind="Internal")

    rg = [list(range(NCORE))]
    out1_flat = out1.ap().rearrange("a (b o) -> (a b) o", o=1)
    kers_r = ker_in.ap().rearrange("(k p) c -> p k c", p=P)  # [128,4,CLOC]
    kg_r = kg_in.ap().rearrange("(k p) c -> p k c", p=P)     # [128,4,256]

    with tile.TileContext(nc) as tc:
        _emit(nc, tc, emb_in, offs_in, ctab_in, eye_in, iota_in, labv_in,
              rowid_in, kers_r, kg_r, out1, out2, dbg,
              arb_i, arb_o, agc_i, agc_o, rb_d, rg, out1_flat)

    nc.compile()
    return nc


def _emit(nc, tc, emb_in, offs_in, ctab_in, eye_in, iota_in, labv_in,
          rowid_in, kers_r, kg_r, out1, out2, dbg,
          arb_i, arb_o, agc_i, agc_o, rb_d, rg, out1_flat):
    with ExitStack() as top:
        cp = top.enter_context(tc.tile_pool(name="const", bufs=1))
        eye = cp.tile([P, P], F32)
        nc.sync.dma_start(eye[:], eye_in.ap())
        ones_k = cp.tile([P, 1], F32)
        nc.vector.memset(ones_k[:], 1.0)
        ones_r = cp.tile([1, P], F32)
        nc.vector.memset(ones_r[:], 1.0)
        ones_kb = cp.tile([P, 1], BF16)
        nc.vector.memset(ones_kb[:], 1.0)
        ones_rb = cp.tile([1, P], BF16)
        nc.vector.memset(ones_rb[:], 1.0)
        ones128b = cp.tile([P, P], BF16)
        nc.vector.memset(ones128b[:], 1.0)
        ctab = cp.tile([16, 32], F32)
        nc.sync.dma_start(ctab[:], ctab_in.ap())
        iotaf = cp.tile([P, CH], F32)
        nc.sync.dma_start(iotaf[:], iota_in.ap())
        labv = cp.tile([P, 2], F32)
        nc.sync.dma_start(labv[:], labv_in.ap())
        rowid = cp.tile([P, 2], F32)
        nc.sync.dma_start(rowid[:], rowid_in.ap())
        offs = cp.tile([P, 2], I32)
        nc.sync.dma_start(offs[:], offs_in.ap())
        embT = [cp.tile([P, N], BF16, name=f"embT_{k}") for k in range(4)]

        # ---------------- phase A: embedding prep --------------------------
        with ExitStack() as s0:
            pp = s0.enter_context(tc.tile_pool(name="prep", bufs=2))
            pps = s0.enter_context(tc.tile_pool(name="prepps", bufs=2,
                                                space="PSUM"))
            for g in range(2):
                et = pp.tile([P, D], F32, tag="et")
                nc.sync.dma_start(et[:], emb_in.ap()[g * P:(g + 1) * P, :])
                sscr = pp.tile([P, D], F32, tag="sscr")
                n2 = pp.tile([P, 1], F32, tag="n2")
                nc.scalar.activation(sscr[:], et[:], AF.Square,
                                     accum_out=n2[:])
                rinv = pp.tile([P, 1], F32, tag="rinv")
                nc.scalar.activation(rinv[:], n2[:], AF.Abs_reciprocal_sqrt)
                et64 = pp.tile([P, D], F32, tag="et64")
                nc.vector.tensor_scalar(et64[:], et[:], rinv[:, :1], SCALE,
                                        op0=OP.mult, op1=OP.mult)
                for k in range(4):
                    pt = pps.tile([P, P], F32)
                    nc.tensor.transpose(pt[:], et64[:, k * P:(k + 1) * P],
                                        eye[:])
                    nc.scalar.activation(
                        embT[k][:, g * P:(g + 1) * P], pt[:], AF.Copy)

        # ---------------- phase A2: early tgt from label columns -----------
        # kg holds, for every row, the kernel column of its label if this
        # core owns it (else a dummy).  Run it through the exact pipeline
        # of the main sweep so the extracted value is bit-identical to the
        # sweep's label element, then AllReduce (overlaps the sweep).
        tgt = cp.tile([P, 2], F32)
        with ExitStack() as sA:
            ap_ = sA.enter_context(tc.tile_pool(name="a2", bufs=1))
            aps = sA.enter_context(tc.tile_pool(name="a2ps", bufs=2,
                                                space="PSUM"))
            kgb = ap_.tile([P, 4, N], BF16)
            nc.sync.dma_start(kgb[:], kg_r[:, :, :])
            sqg = ap_.tile([P, 4, N], BF16, tag="sqg")
            nc.scalar.activation(sqg[:], kgb[:], AF.Square)
            ksg = ap_.tile([P, 2, N], BF16, tag="ksg")
            nc.vector.tensor_tensor(ksg[:, 0, :], sqg[:, 0, :], sqg[:, 1, :],
                                    OP.add)
            nc.vector.tensor_tensor(ksg[:, 1, :], sqg[:, 2, :], sqg[:, 3, :],
                                    OP.add)
            ksgs = ap_.tile([P, N], BF16, tag="ksgs")
            nc.vector.tensor_tensor(ksgs[:], ksg[:, 0, :], ksg[:, 1, :],
                                    OP.add)
            png = aps.tile([P, N], F32)
            nc.tensor.matmul(png[:], ones128b[:], ksgs[:],
                             start=True, stop=True)
            rsg = ap_.tile([P, N], F32, tag="rsg")
            nc.scalar.activation(rsg[:], png[:], AF.Abs_reciprocal_sqrt)
            tloc = ap_.tile([P, 2], F32, tag="tloc")
            for m in range(2):
                pcg = aps.tile([P, N], F32, tag="pcg")
                for k in range(4):
                    nc.tensor.matmul(pcg[:], embT[k][:, m * P:(m + 1) * P],
                                     kgb[:, k, :], start=(k == 0),
                                     stop=(k == 3))
                slg = ap_.tile([P, N], F32, tag="slg")
                nc.vector.tensor_tensor(slg[:], pcg[:], rsg[:], OP.mult)
                scr = ap_.tile([P, N], F32, tag="scrg")
                nc.vector.scalar_tensor_tensor(
                    scr[:], iotaf[:, 0:N], rowid[:, m:m + 1], slg[:],
                    op0=OP.is_equal, op1=OP.mult,
                    accum_out=tloc[:, m:m + 1])
            nc.sync.dma_start(arb_i.ap(), tloc[:])
            nc.gpsimd.collective_compute(
                "AllReduce", OP.add, replica_groups=rg,
                ins=[arb_i.ap()], outs=[arb_o.ap()])
            nc.sync.dma_start(tgt[:], arb_o.ap())
        negtgt = cp.tile([P, 2], F32)
        nc.vector.tensor_scalar(negtgt[:], tgt[:], -1.0, None, op0=OP.mult)

        # ---------------- phase B: fused main sweep ------------------------
        # per chunk: cast-load kernel cols, col rsqrt via ln/exp, cos
        # matmul, write both outputs, then count/shift/max8 immediately.
        cnt = cp.tile([P, 2, NCH], F32)
        cand0 = cp.tile([P, 2, NCH * 8], F32)
        with ExitStack() as s1:
            kp = s1.enter_context(tc.tile_pool(name="kt", bufs=3))
            sqp = s1.enter_context(tc.tile_pool(name="sq", bufs=3))
            lnp = s1.enter_context(tc.tile_pool(name="ln", bufs=3))
            rsp = s1.enter_context(tc.tile_pool(name="rs", bufs=3))
            slp = s1.enter_context(tc.tile_pool(name="sl", bufs=4))
            msp = s1.enter_context(tc.tile_pool(name="msh", bufs=3))
            pcp = s1.enter_context(tc.tile_pool(name="pc", bufs=6,
                                                space="PSUM"))
            pnp = s1.enter_context(tc.tile_pool(name="pn", bufs=2,
                                                space="PSUM"))
            for ci in range(NCH):
                c0 = ci * CH
                ktb = kp.tile([P, 4, CH], BF16)
                nc.sync.dma_start(ktb[:], kers_r[:, :, c0:c0 + CH])
                pcs = []
                for m in range(2):
                    pcos = pcp.tile([P, CH], F32)
                    for k in range(4):
                        nc.tensor.matmul(pcos[:],
                                         embT[k][:, m * P:(m + 1) * P],
                                         ktb[:, k, :],
                                         start=(k == 0), stop=(k == 3))
                    pcs.append(pcos)
                sqt = sqp.tile([P, 4, CH], BF16)
                nc.scalar.activation(sqt[:], ktb[:], AF.Square)
                ks2 = sqp.tile([P, 2, CH], BF16, tag="ks2")
                nc.vector.tensor_tensor(ks2[:, 0, :], sqt[:, 0, :],
                                        sqt[:, 1, :], OP.add)
                nc.vector.tensor_tensor(ks2[:, 1, :], sqt[:, 2, :],
                                        sqt[:, 3, :], OP.add)
                ks = sqp.tile([P, CH], BF16, tag="ks")
                nc.vector.tensor_tensor(ks[:], ks2[:, 0, :], ks2[:, 1, :],
                                        OP.add)
                pnrm = pnp.tile([P, CH], F32)
                nc.tensor.matmul(pnrm[:], ones128b[:], ks[:],
                                 start=True, stop=True)
                rs = rsp.tile([P, CH], F32)
                nc.scalar.activation(rs[:], pnrm[:], AF.Abs_reciprocal_sqrt)
                sl = slp.tile([P, 2, CH], F32)
                sg = msp.tile([P, 2, CH], BF16, tag="sg")
                for m in range(2):
                    pcos = pcs[m]
                    nc.vector.tensor_tensor(sl[:, m, :], pcos[:], rs[:],
                                            OP.mult)
                    nc.sync.dma_start(
                        out2.ap()[m * P:(m + 1) * P, c0:c0 + CH], sl[:, m, :])
                    nc.sync.dma_start(
                        out1.ap()[m * P:(m + 1) * P, c0:c0 + CH], sl[:, m, :])
                    nc.scalar.activation(sg[:, m, :], sl[:, m, :], AF.Sign,
                                         bias=negtgt[:, m:m + 1],
                                         accum_out=cnt[:, m, ci:ci + 1])
                sh = msp.tile([P, 2, CH], F32, tag="sh")
                nc.vector.scalar_tensor_tensor(
                    sh[:], sg[:], 0.0, sl[:], op0=OP.is_lt, op1=OP.mult)
                for m in range(2):
                    nc.vector.max(cand0[:, m, ci * 8:ci * 8 + 8],
                                  sh[:, m, :])

        sm = top.enter_context(tc.tile_pool(name="small", bufs=1))
        sps = top.enter_context(tc.tile_pool(name="smallps", bufs=1,
                                             space="PSUM"))

        # ---------------- phase E: counts + sorted top-32 + AG -------------
        # count_gt = count_ge - #owned labels (evicted bit-exactly)
        own = sm.tile([P, 2], F32)
        nc.vector.tensor_scalar(own[:], labv[:], -0.5, None, op0=OP.is_gt)
        owns = sm.tile([P, 1], F32)
        nc.vector.tensor_reduce(owns[:], own[:], AX.X, OP.add)
        cntr = sm.tile([P, 1], F32)
        nc.vector.tensor_reduce(cntr[:], cnt[:].rearrange("p a b -> p (a b)"),
                                AX.X, OP.add)
        nc.vector.tensor_scalar(cntr[:], cntr[:], float(2 * CLOC), None,
                                op0=OP.add)
        nc.vector.tensor_tensor(cntr[:], cntr[:], owns[:], OP.subtract)
        nc.vector.tensor_scalar(cntr[:], cntr[:], 0.5, None, op0=OP.mult)
        pcnt = sps.tile([1, 1], F32, tag="pcnt")
        nc.tensor.matmul(pcnt[:], cntr[:], ones_k[:], start=True, stop=True)
        cnts = sm.tile([1, 1], F32)
        nc.scalar.activation(cnts[:], pcnt[:], AF.Copy)

        # per (partition, group): 4 rounds of max8 + match_replace over the
        # 200 chunk-candidates -> sorted top-32 list L
        L = sm.tile([P, 2, LTOP], F32)
        for g in range(2):
            blk = cand0[:, g, :]
            for r in range(4):
                nc.vector.max(L[:, g, 8 * r:8 * (r + 1)], blk)
                if r < 3:
                    nc.vector.match_replace(blk, L[:, g, 8 * r:8 * (r + 1)],
                                            blk, NEG_PAD)

        # ship L + count partial in one AllGather
        lflat = agc_i.ap()[0:1, 0:P * GW].rearrange("o (p f) -> (o p) f", p=P)
        nc.sync.dma_start(lflat, L[:].rearrange("p a b -> p (a b)"))
        nc.sync.dma_start(agc_i.ap()[0:1, P * GW:P * GW + 1], cnts[:])
        nc.gpsimd.collective_compute(
            "AllGather", OP.bypass, replica_groups=rg,
            ins=[agc_i.ap()], outs=[agc_o.ap()])

        # ---------------- phase F: gather back, far_rank, transpose --------
        G = sm.tile([P, GALL], F32)          # col r*64 + g*32 + j
        for r in range(NCORE):
            blk = agc_o.ap()[r:r + 1, 0:P * GW].rearrange(
                "o (p f) -> (o p) f", p=P)
            nc.sync.dma_start(G[:, r * GW:(r + 1) * GW], blk)
        cntg = sm.tile([1, NCORE], F32)
        nc.sync.dma_start(cntg[:], agc_o.ap()[:, P * GW:P * GW + 1]
                          .rearrange("a o -> o a"))
        tsum = sm.tile([1, 1], F32)
        nc.vector.tensor_reduce(tsum[:], cntg[:], AX.X, OP.add)
        a_t = sm.tile([1, 1], F32)
        nc.vector.tensor_scalar(a_t[:], tsum[:], -1.0, float(NC1),
                                op0=OP.mult, op1=OP.add)

        # k_idx = clip(far_rank - 1, 0, 255) via counting 99999*k < A
        pa16 = sps.tile([16, 1], F32, tag="pa16")
        nc.tensor.matmul(pa16[:], ones_r[0:1, 0:16], a_t[:],
                         start=True, stop=True)
        a16 = sm.tile([16, 1], F32)
        nc.scalar.activation(a16[:], pa16[:], AF.Copy)
        kscr = sm.tile([16, 16], F32)
        kpart = sm.tile([16, 1], F32)
        nc.vector.tensor_scalar(kscr[:], ctab[:, 0:16], a16[:, :1], None,
                                op0=OP.is_lt, op1=OP.add,
                                accum_out=kpart[:])
        pki = sps.tile([1, 1], F32, tag="pki")
        nc.tensor.matmul(pki[:], kpart[:], ones_k[0:16, :],
                         start=True, stop=True)
        ki = sm.tile([1, 1], F32)
        nc.scalar.activation(ki[:], pki[:], AF.Copy)
        pki128 = sps.tile([P, 1], F32, tag="pki128")
        nc.tensor.matmul(pki128[:], ones_r[:], ki[:], start=True, stop=True)
        ki128 = sm.tile([P, 1], F32)
        nc.scalar.activation(ki128[:], pki128[:], AF.Copy)

        # transpose G so sorted-rank slots become partitions
        T = sm.tile([P, GALL], F32)
        with ExitStack() as s3:
            tps = s3.enter_context(tc.tile_pool(name="tp", bufs=2,
                                                space="PSUM"))
            for b in range(4):
                pt = tps.tile([P, P], F32)
                nc.tensor.transpose(pt[:], G[:, b * P:(b + 1) * P], eye[:])
                nc.scalar.activation(T[:, b * P:(b + 1) * P], pt[:], AF.Copy)

        # final candidates: top-8 of each transposed half (cores 0-3 / 4-7)
        fin = sm.tile([P, 2, 8], F32)
        for h in range(2):
            nc.vector.max(fin[:, h, :], T[:, 256 * h:256 * (h + 1)])

        # ---------------- phase G: exact th via rank counting --------------
        # top-4 of each half per partition = 1024 values containing the
        # global top-400.  Broadcast them to every partition, rank each by
        # an is_gt count, and select the one whose rank == k_idx.
        f8c = sm.tile([P, 2, 4], F32)            # contiguous top-4 per half
        nc.vector.tensor_scalar(f8c[:], fin[:, :, 0:4], 0.0, None, op0=OP.add)
        rb_flat = rb_d.ap()[0:1, :].rearrange("o (p f) -> (o p) f", p=P)
        nc.sync.dma_start(rb_flat, f8c[:].rearrange("p a b -> p (a b)"))
        row = sm.tile([1, RW], F32)
        nc.sync.dma_start(row[:], rb_d.ap())
        R = sm.tile([P, RW], F32)
        with ExitStack() as s4:
            prp = s4.enter_context(tc.tile_pool(name="prp", bufs=1,
                                                space="PSUM"))
            pR = prp.tile([P, RW], F32)
            for hh in range(2):
                nc.tensor.matmul(pR[:, hh * 512:(hh + 1) * 512], ones_r[:],
                                 row[:, hh * 512:(hh + 1) * 512],
                                 start=True, stop=True)
            nc.scalar.activation(R[:], pR[:], AF.Copy)
        rnk = sm.tile([P, 2, 4], F32)
        rscr = sm.tile([P, RW], F32)
        for h in range(2):
            for j in range(4):
                nc.vector.tensor_scalar(rscr[:], R[:], f8c[:, h, j:j + 1],
                                        None, op0=OP.is_gt, op1=OP.add,
                                        accum_out=rnk[:, h, j:j + 1])
        m8 = sm.tile([P, 2, 4], F32)
        nc.vector.tensor_scalar(m8[:], rnk[:], ki128[:, :1], None,
                                op0=OP.is_equal)
        selv = sm.tile([P, 2, 4], F32)
        nc.vector.tensor_tensor(selv[:], m8[:], f8c[:], OP.mult)
        thp = sm.tile([P, 1], F32)
        nc.vector.tensor_reduce(thp[:], selv[:].rearrange("p a b -> p (a b)"),
                                AX.X, OP.add)
        pth = sps.tile([1, 1], F32, tag="pth")
        nc.tensor.matmul(pth[:], thp[:], ones_k[:], start=True, stop=True)
        th1 = sm.tile([1, 1], F32)
        nc.scalar.activation(th1[:], pth[:], AF.Copy)
        pth128 = sps.tile([P, 1], F32, tag="pth128")
        nc.tensor.matmul(pth128[:], ones_r[:], th1[:], start=True, stop=True)
        th128 = sm.tile([P, 1], F32)
        nc.scalar.activation(th128[:], pth128[:], AF.Copy)

        # ---------------- phase H: moments from candidates -----------------
        sq = sm.tile([P, 2], F32)
        tm = sm.tile([P, 2], F32)
        wsc = sm.tile([P, GALL], F32)
        w2 = sm.tile([P, GALL], F32)
        for g in range(2):
            Gg = G[:].rearrange("p (r g j) -> p g r j", g=2, j=LTOP)[:, g, :, :]
            wv = wsc[:].rearrange("p (r g j) -> p g r j",
                                  g=2, j=LTOP)[:, g, :, :]
            w2v = w2[:].rearrange("p (r g j) -> p g r j",
                                  g=2, j=LTOP)[:, g, :, :]
            nc.vector.scalar_tensor_tensor(
                wv, Gg, th128[:, :1], Gg, op0=OP.is_gt, op1=OP.mult)
            nc.scalar.activation(w2v, wv, AF.Square, accum_out=sq[:, g:g + 1])
            nc.vector.tensor_scalar(wv, Gg, th128[:, :1], None,
                                    op0=OP.is_gt, op1=OP.add,
                                    accum_out=tm[:, g:g + 1])

        # ---------------- phase I: final scalar math + patch out1 ----------
        times = sm.tile([P, 2], F32)
        nc.vector.tensor_scalar(times[:], tm[:], 1.0, None, op0=OP.max)
        rec = sm.tile([P, 2], F32)
        nc.vector.reciprocal(rec[:], times[:])
        nm = sm.tile([P, 2], F32)
        nc.vector.tensor_tensor(nm[:], sq[:], rec[:], OP.mult)
        nc.vector.tensor_scalar(nm[:], nm[:], 1.0 / (SCALE * SCALE), None,
                                op0=OP.mult)
        x5 = sm.tile([P, 2], F32)
        nc.vector.tensor_scalar(x5[:], tgt[:], SCALE, None, op0=OP.add)
        x6 = sm.tile([P, 2], F32)
        nc.vector.tensor_tensor(x6[:], x5[:], nm[:], OP.mult)
        pv2 = sm.tile([P, 2], F32)
        nc.vector.tensor_tensor(pv2[:], tgt[:], x6[:], OP.subtract)
        nc.vector.tensor_scalar(pv2[:], pv2[:], -SCALE * MARGIN, None,
                                op0=OP.add)
        for g in range(2):
            nc.gpsimd.indirect_dma_start(
                out=out1_flat,
                out_offset=bass.IndirectOffsetOnAxis(ap=offs[:, g:g + 1],
                                                     axis=0),
                in_=pv2[:, g:g + 1], in_offset=None,
                bounds_check=N * CLOC - 1, oob_is_err=False)

        nc.sync.dma_start(dbg.ap()[:, 0:2], sq[:])
        nc.sync.dma_start(dbg.ap()[:, 2:4], tm[:])
        nc.sync.dma_start(dbg.ap()[:, 6:7], th128[:])
        nc.sync.dma_start(dbg.ap()[:, 7:9], nm[:])
        nc.sync.dma_start(dbg.ap()[:, 11:13], pv2[:])
        nc.sync.dma_start(dbg.ap()[:, 13:15], tgt[:])
        nc.sync.dma_start(dbg.ap()[0:1, 15:16], ki[0:1, :])
        nc.sync.dma_start(dbg.ap()[0:1, 4:5], cnts[:])


_NC = None


def _get_nc():
    global _NC
    if _NC is None:
        _NC = build()
    return _NC


def _make_in_maps(embeddings, kernel, label):
    emb = np.ascontiguousarray(np.asarray(embeddings, dtype=np.float32))
    ker = np.asarray(kernel, dtype=np.float32)
    lab = np.asarray(label).astype(np.int64)

    ctab = np.zeros((16, 32), np.float32)
    kk = (np.arange(16)[:, None] * 16 + np.arange(16)[None, :])
    ctab[:, :16] = (float(C - 1) * kk).astype(np.float32)
    ctab[0, 0] = 1.0e30
    ctab[:, 16:] = kk.astype(np.float32)
    eye = np.eye(P, dtype=np.float32)
    iotaf = np.tile(np.arange(CH, dtype=np.float32), (P, 1))

    rows = np.arange(N)
    in_maps = []
    for c in range(NCORE):
        loc = lab - c * CLOC
        owned = (loc >= 0) & (loc < CLOC)
        off = np.where(owned, rows * CLOC + loc, BIGOFF).astype(np.int32)
        offs = off.reshape(2, P).T.copy()  # [128, 2]: row i = p + 128*g
        labv = np.where(owned, loc, -5.0).astype(np.float32)
        labv = labv.reshape(2, P).T.copy()  # [128, 2]
        rowid = np.where(owned, rows, -5.0).astype(np.float32)
        rowid = rowid.reshape(2, P).T.copy()  # [128, 2]
        kslice = np.ascontiguousarray(
            ker[:, c * CLOC:(c + 1) * CLOC].astype(ml_dtypes.bfloat16))
        kg = kslice[:, np.where(owned, loc, 0)]  # [512, 256] bf16
        in_maps.append({
            "embeddings": emb,
            "kers": kslice,
            "kg": np.ascontiguousarray(kg),
            "offs": offs,
            "ctab": ctab,
            "eye": eye,
            "iotaf": iotaf,
            "labv": labv,
            "rowid": rowid,
        })
    return in_maps


def run(embeddings, kernel, label, trace=False):
    nc = _get_nc()
    in_maps = _make_in_maps(embeddings, kernel, label)
    res = bass_utils.run_bass_kernel_spmd(
        nc, in_maps, core_ids=list(range(NCORE)), trace=trace)
    out1 = np.concatenate([res.results[c]["out1"] for c in range(NCORE)],
                          axis=1)
    out2 = np.concatenate([res.results[c]["out2"] for c in range(NCORE)],
                          axis=1)
    return (out1, out2), res


def kernel(**inputs):
    outs, _ = run(inputs["embeddings"], inputs["kernel"], inputs["label"])
    return outs
